# revision 23
# baseline (speedup 1.0000x reference)
"""Causal single-head self-attention on 8 TRN2 NeuronCores.

Sharding: 8 cores = 4 batches x 2 cores/batch. Within a batch the 8
512-query chunks are split zigzag (core A owns chunks {0,3,4,7}, core B
{1,2,5,6}) so causal work balances (18 units each). Each core projects
K/V for the whole batch from its own copy of x (recompute beats
cross-core K/V exchange at this size), computes Q only for its owned
chunks, then does block-causal flash-style attention without the
row-max pass (scores here are O(1) so exp never overflows) and a fused
out-projection.

SPMD trick: one program runs on all 8 cores, so per-core differences
live in the DATA only. x rows are fed in a per-core storage permutation
that puts each core's owned query chunks at uniform offsets (storage
chunks 0,2,4,6). Causal masking splits into a role-independent part and
a data part: the diagonal k-chunk (storage chunk 2g of slot g) gets a
compile-time triangular mask (slices of one "staircase" tile) applied
over the valid q-suffix only; the other boundary chunk is all-valid or
all-masked per core, folded into the exp as a per-partition bias of
0/-30 (exp(-30)~=0) -- no per-core mask tensors at all.

x is passed D-major AND bf16 (cast on host: halves the HBM read); the
output is written bf16 (upcast on host). All transposes run on the
tensor engine.

Pipelining: emission is woven so the in-order engine queues interleave
next-chunk K/V/Q projection matmuls into the exp-wait bubbles of the
attention block stream; PSUM is one shared 1-bank pool (scores /
projections / transposes / out-proj, 4 banks) + 4 rotating po banks.

Layouts (partition dim first):
  xT   [128, 8, 4096]  bf16   x^T per d-chunk
  K^T  [128, 4096]     bf16   H-major keys
  Q^T  [128, 2048]     bf16   H-major owned queries
  V    [128, 32, 256]  bf16   token-major V tiles (PE-transposed from the
                              H-major projection); col 128 = ones column
                              for the fused rowsum trick
  scores_T [k=128, q<=512] PSUM; P_T = exp(scale*s + bias) bf16 (ACT)
  O [q=128, 128+1] accumulates in PSUM over k-blocks with P_T subtiles as
  the stationary operand and [V|1] moving; col 128 = softmax denominator.
  Out-proj: lhsT = O^T tile (PE transpose), rhs = Wo^T; the 1/denominator
  scale rides the PSUM->SBUF copy (tensor_scalar_mul on DVE for one half,
  ACT copy-with-scale for the other).
"""

import numpy as np
import ml_dtypes
from contextlib import ExitStack

import concourse.bass as bass
import concourse.tile as tile
from concourse import bacc, mybir
from concourse.bass_utils import run_bass_kernel_spmd

S, B, D, H = 4096, 4, 1024, 128
P = 128
QC = 512                  # query chunk
NSLOT = 4                 # owned chunks per core
DC = D // P               # 8 d-chunks
TT = S // P               # 32 token tiles / k-blocks
NKT = S // QC             # 8 key 512-chunks
SCALE = float(H) ** -0.5
UW = 896                  # staircase width: 384 zero-pad + 512 triangle

# storage-order permutation of the 8 query chunks, per role. Queries the
# core owns sit at storage chunks 0,2,4,6; the first 2(g+1) storage
# chunks cover every true key needed by owned chunk g (extras masked).
SIGMA = {0: [0, 1, 3, 2, 4, 5, 7, 6], 1: [1, 0, 2, 3, 5, 4, 6, 7]}
QSLOT = [0, 2, 4, 6]      # storage chunk positions of owned queries

F32 = mybir.dt.float32
BF16 = mybir.dt.bfloat16


def _build_kernel():
    nc = bacc.Bacc("TRN2", target_bir_lowering=False, debug=False, num_devices=8)

    xbT = nc.dram_tensor("xbT", [D, S], BF16, kind="ExternalInput")
    wqT = nc.dram_tensor("wqT", [P, DC, H], BF16, kind="ExternalInput")
    wkT = nc.dram_tensor("wkT", [P, DC, H], BF16, kind="ExternalInput")
    wvT = nc.dram_tensor("wvT", [P, DC, H], BF16, kind="ExternalInput")
    woT = nc.dram_tensor("woT", [H, D], BF16, kind="ExternalInput")
    bias = nc.dram_tensor("bias", [P, NSLOT * 8], F32, kind="ExternalInput")
    out = nc.dram_tensor("out", [NSLOT * QC, D], BF16, kind="ExternalOutput")

    with ExitStack() as ctx:
        tc = ctx.enter_context(tile.TileContext(nc))
        _body(ctx, tc, xbT.ap(), wqT.ap(), wkT.ap(), wvT.ap(), woT.ap(),
              bias.ap(), out.ap())

    nc.compile()
    return nc


def _interleave(a_units, b_units):
    """Emit a-units with b-units spread evenly between them (b slightly
    front-loaded so woven work finishes before the a-stream does)."""
    if not a_units:
        for b in b_units:
            b()
        return
    ratio = len(b_units) / len(a_units)
    acc = 0.0
    bi = 0
    for a in a_units:
        a()
        acc += ratio
        while bi < len(b_units) and acc >= 1.0:
            b_units[bi]()
            bi += 1
            acc -= 1.0
    while bi < len(b_units):
        b_units[bi]()
        bi += 1


def _body(ctx, tc, xbT, wqT, wkT, wvT, woT, bias, out):
    nc = tc.nc

    consts = ctx.enter_context(tc.tile_pool(name="consts", bufs=1))
    bigbuf = ctx.enter_context(tc.tile_pool(name="bigbuf", bufs=1))
    ptpool = ctx.enter_context(tc.tile_pool(name="pt", bufs=8))
    otmp_pool = ctx.enter_context(tc.tile_pool(name="otmp", bufs=6))
    ypool = ctx.enter_context(tc.tile_pool(name="y", bufs=4))
    # One shared 1-bank pool for scores / projections / transposes /
    # out-proj (4 banks) + 4 rotating po accumulator banks = 8 banks.
    pspool = ctx.enter_context(tc.tile_pool(name="ps", bufs=6, space="PSUM"))
    psO = ctx.enter_context(tc.tile_pool(name="psO", bufs=1, space="PSUM"))

    # ---- constants (cast f32 -> bf16 in the SWDGE DMA) ----
    wq_sb = consts.tile([P, DC, H], BF16)
    wk_sb = consts.tile([P, DC, H], BF16)
    wv_sb = consts.tile([P, DC, H], BF16)
    woT_sb = consts.tile([P, D], BF16)
    bias_sb = consts.tile([P, NSLOT * 8], F32)

    xT = bigbuf.tile([P, DC, S], BF16)
    k_sb = bigbuf.tile([P, S], BF16)
    vT_sb = bigbuf.tile([P, S], BF16)
    q_sb = bigbuf.tile([P, NSLOT * QC], BF16)
    v_sb = bigbuf.tile([P, TT, 2 * P], BF16)  # V k-blocks + ones col (padded stride)
    o_t = bigbuf.tile([P, NSLOT * NSLOT, P], BF16)  # O^T [h, q-tile, q], unnorm
    rec_sb = bigbuf.tile([P, NSLOT * NSLOT], F32)   # 1/rowsum per q-tile column

    nc.gpsimd.dma_start(wk_sb[:], wkT)
    nc.sync.dma_start(wv_sb[:], wvT)
    nc.sync.dma_start(wq_sb[:], wqT)
    # x chunk 0 then chunk 1, one [128, 512] DMA per d-chunk, issue
    # streams striped across the gpsimd and sync queues so descriptor
    # generation parallelizes (projection chains consume c in order).
    for kt in range(2):
        for c in range(DC):
            (nc.gpsimd if c % 2 == 0 else nc.sync).dma_start(
                xT[:, c, bass.ts(kt, QC)], xbT[bass.ts(c, P), bass.ts(kt, QC)])
        if kt == 0:
            nc.gpsimd.dma_start(bias_sb[:], bias)
    nc.gpsimd.dma_start(woT_sb[:], woT)
    # stream the rest of x up front: the DGE queue drains continuously
    # instead of bursting at slot boundaries (consumers dep on pair DMAs)
    for c0 in range(2, NKT, 2):
        for c in range(DC):
            nc.gpsimd.dma_start(xT[:, c, bass.ds(c0 * QC, 2 * QC)],
                                xbT[bass.ts(c, P), bass.ds(c0 * QC, 2 * QC)])

    # Staircase tile: univ[k, x] = 1.0 where x - 384 >= k. The triangular
    # mask for diagonal k-block j is univ[:, 384 : 896-128j] over the
    # valid q-suffix, and the 128x128 identity (for PE transposes) is
    # univ[:, 384:512] - univ[:, 383:511].
    stair = consts.tile([P, UW], F32)
    nc.gpsimd.iota(stair[:], pattern=[[1, UW]], base=-384, channel_multiplier=-1,
                   allow_small_or_imprecise_dtypes=True)
    univ = consts.tile([P, UW], BF16)
    nc.vector.tensor_scalar(univ[:], stair[:], 0.0, None,
                            op0=mybir.AluOpType.is_ge)
    ident = consts.tile([P, P], BF16)
    nc.vector.tensor_sub(ident[:], univ[:, 384 : 384 + P], univ[:, 383 : 383 + P])
    nc.vector.memset(v_sb[:, :, H], 1.0)  # ones column for rowsum trick

    # ---------- emission units ----------
    def proj_units(w_sb, dst, src_kt, dst_kt=None, pool=None, pname="ps"):
        """Project one 512-token chunk through one weight: 4 units of
        2 accumulating matmuls each + a final PSUM->SBUF copy."""
        st = {}
        dkt = src_kt if dst_kt is None else dst_kt
        ppool = pool if pool is not None else pspool

        def unit(i):
            def run():
                if i == 0:
                    st["ps"] = ppool.tile([P, QC], F32, name=pname)
                for c in (2 * i, 2 * i + 1):
                    nc.tensor.matmul(st["ps"][:], lhsT=w_sb[:, c, :],
                                     rhs=xT[:, c, bass.ts(src_kt, QC)],
                                     start=(c == 0), stop=(c == DC - 1))
                if i == 3:
                    nc.vector.tensor_copy(dst[:, bass.ts(dkt, QC)], st["ps"][:])
            return run
        return [unit(i) for i in range(4)]

    def vtr_unit(bk):
        def run():
            pstr = pspool.tile([P, P], BF16, name="ps")
            nc.tensor.transpose(pstr[:], vT_sb[:, bass.ts(bk, P)], ident[:])
            nc.vector.tensor_copy(v_sb[:, bk, 0:H], pstr[:])
        return run

    po_tiles = {}  # slot -> (po3 bank: subs 0-2, po1 bank: sub 3)

    def po_ap(g, sub):
        po3, po1 = po_tiles[g]
        return po3[:, sub, :] if sub < 3 else po1[:]

    def blk_unit(g, bk, nb):
        def run():
            if bk == 0:
                po_tiles[g] = (psO.tile([P, 3, H + 1], F32, name="po3"),
                               psO.tile([P, H + 1], F32, name="po1"))
            p = bk - 8 * g  # boundary sub-index (>=0 for the last 8 blocks)
            diag_j = p if 0 <= p <= 3 else None
            q0 = 128 * diag_j if diag_j else 0  # valid q-suffix start
            qw = QC - q0
            ps = pspool.tile([P, QC], F32, name="ps")
            nc.tensor.matmul(ps[:, q0:], lhsT=k_sb[:, bass.ts(bk, P)],
                             rhs=q_sb[:, g * QC + q0 : (g + 1) * QC],
                             start=True, stop=True)
            pt = ptpool.tile([P, QC], BF16)
            bias_ap = 0.0
            if p >= 4:  # off-diagonal boundary chunk: all-or-nothing per core
                bias_ap = bias_sb[:, g * 8 + p : g * 8 + p + 1]
            nc.scalar.activation(pt[:, q0:], ps[:, q0:],
                                 mybir.ActivationFunctionType.Exp,
                                 bias=bias_ap, scale=SCALE)
            if diag_j is not None:  # compile-time triangle over the suffix
                nc.vector.tensor_mul(pt[:, q0:], pt[:, q0:],
                                     univ[:, 384 : UW - 128 * diag_j])
            # Three accumulation groups share the po3 bank: sub 0's
            # start=True clears the whole bank's has_written bits, so
            # subs 1,2 start with start=False (overwrite-on-clear-bit).
            for sub in range(diag_j or 0, NSLOT):
                nc.tensor.matmul(po_ap(g, sub), lhsT=pt[:, bass.ts(sub, P)],
                                 rhs=v_sb[:, bk, 0 : H + 1],
                                 start=(bk == 0 and sub in (0, 3)),
                                 stop=(bk == nb - 1),
                                 skip_group_check=(sub in (1, 2)))
        return run

    def finish_unit(g, sub):
        def run():
            po = po_ap(g, sub)
            idx = g * NSLOT + sub
            nc.vector.reciprocal(rec_sb[:, idx : idx + 1], po[:, H : H + 1])
            ob = otmp_pool.tile([P, P], BF16, name="ob")
            nc.vector.tensor_copy(ob[:], po[:, 0:H])
            pstr = pspool.tile([P, P], BF16, name="ps")
            nc.tensor.transpose(pstr[:], ob[:], ident[:])
            nc.vector.tensor_copy(o_t[:, idx, :], pstr[:])
        return run

    def outproj_unit(g, sub):
        tt = g * NSLOT + sub

        def run():
            y = ypool.tile([P, D], BF16)
            for half in range(2):
                ps = pspool.tile([P, QC], F32, name="ps")
                nc.tensor.matmul(ps[:], lhsT=o_t[:, tt, :],
                                 rhs=woT_sb[:, bass.ts(half, QC)],
                                 start=True, stop=True)
                if half == 0:  # split the normalize-copies across DVE and ACT
                    nc.vector.tensor_scalar_mul(y[:, bass.ts(half, QC)], ps[:],
                                                rec_sb[:, tt : tt + 1])
                else:
                    nc.scalar.mul(y[:, bass.ts(half, QC)], ps[:],
                                  rec_sb[:, tt : tt + 1])
                nc.sync.dma_start(out[bass.ts(tt, P), bass.ts(half, QC)],
                                  y[:, bass.ts(half, QC)])
        return run

    # ---------- prologue: chunks 0,1 projected; Q for slot 0 ----------
    # Five projection chains drained c-major so each chain consumes every
    # d-chunk right as its DMA lands (the 5th chain borrows a psO bank).
    chains = [proj_units(wk_sb, k_sb, 0),
              proj_units(wv_sb, vT_sb, 0),
              proj_units(wq_sb, q_sb, 0, dst_kt=0),
              proj_units(wk_sb, k_sb, 1),
              proj_units(wv_sb, vT_sb, 1, pool=psO, pname="po3")]
    for j in range(4):
        for ch in chains:
            ch[j]()
    for bk in range(8):
        vtr_unit(bk)()

    # ---------- main: slot-major weave ----------
    for g in range(NSLOT):
        c0 = 2 * g + 2  # chunk pair projected during this slot (preloaded)
        nb = 8 * (g + 1)
        a_units = [blk_unit(g, bk, nb) for bk in range(nb)]
        b_units = []
        if g > 0:
            b_units += [finish_unit(g - 1, s) for s in range(NSLOT)]
            b_units += [outproj_unit(g - 1, s) for s in range(NSLOT)]
        if c0 < NKT:
            b_units += (proj_units(wk_sb, k_sb, c0)
                        + proj_units(wv_sb, vT_sb, c0)
                        + [vtr_unit(bk) for bk in range(4 * c0, 4 * c0 + 4)]
                        + proj_units(wq_sb, q_sb, c0, dst_kt=g + 1)
                        + proj_units(wk_sb, k_sb, c0 + 1)
                        + proj_units(wv_sb, vT_sb, c0 + 1)
                        + [vtr_unit(bk) for bk in range(4 * c0 + 4, 4 * c0 + 8)])
        _interleave(a_units, b_units)

    for s in range(NSLOT):
        finish_unit(NSLOT - 1, s)()
    for s in range(NSLOT):
        outproj_unit(NSLOT - 1, s)()


_CACHED_NC = None


def _get_nc():
    global _CACHED_NC
    if _CACHED_NC is None:
        _CACHED_NC = _build_kernel()
    return _CACHED_NC


def _make_core_inputs(x, wqT, wkT, wvT, woT, core):
    b, role = core // 2, core % 2
    sigma = SIGMA[role]
    perm = np.concatenate([np.arange(QC) + c * QC for c in sigma])
    xbT = np.ascontiguousarray(x[perm, b, :].T.astype(ml_dtypes.bfloat16))

    # exp bias per boundary block: diagonal chunk (p<4) is handled by the
    # compile-time triangle; the other boundary chunk (p>=4) is all-valid
    # (0) or all-masked (-30 => exp~=0) depending on the role's zigzag.
    bias = np.zeros((P, NSLOT * 8), np.float32)
    for g in range(NSLOT):
        c_q = sigma[2 * g]
        c_k = sigma[2 * g + 1]
        if c_k > c_q:
            bias[:, g * 8 + 4 : g * 8 + 8] = -30.0
    return {"xbT": xbT, "wqT": wqT, "wkT": wkT, "wvT": wvT, "woT": woT,
            "bias": bias}


def _w_pch(w):
    """(H, D) weight -> [p, c, h] bf16 layout for a contiguous SBUF load."""
    return np.ascontiguousarray(
        np.asarray(w, np.float32).T.reshape(DC, P, H).transpose(1, 0, 2)
        .astype(ml_dtypes.bfloat16))


def kernel(x, Wq, Wk, Wv, Wo):
    x = np.asarray(x, dtype=np.float32)
    wqT = _w_pch(Wq)
    wkT = _w_pch(Wk)
    wvT = _w_pch(Wv)
    woT = np.ascontiguousarray(np.asarray(Wo, np.float32).T.astype(ml_dtypes.bfloat16))

    nc = _get_nc()
    in_maps = [_make_core_inputs(x, wqT, wkT, wvT, woT, i) for i in range(8)]
    res = run_bass_kernel_spmd(nc, in_maps, list(range(8))).results

    out = np.empty((S, B, D), np.float32)
    for core in range(8):
        b, role = core // 2, core % 2
        sigma = SIGMA[role]
        co = np.asarray(res[core]["out"], np.float32)
        for g in range(NSLOT):
            c_g = sigma[QSLOT[g]]
            out[c_g * QC : (c_g + 1) * QC, b, :] = co[g * QC : (g + 1) * QC, :]
    return out


# revision 24
# speedup vs baseline: 1.0930x; 1.0930x over previous
"""Causal single-head self-attention on 8 TRN2 NeuronCores.

Sharding: 8 cores = 4 batches x 2 cores/batch. Within a batch the 8
512-query chunks are split zigzag (core A owns chunks {0,3,4,7}, core B
{1,2,5,6}) so causal work balances (18 units each). Each core projects
K/V for the whole batch from its own copy of x (recompute beats
cross-core K/V exchange at this size), computes Q only for its owned
chunks, then does block-causal flash-style attention without the
row-max pass (scores here are O(1) so exp never overflows) and a fused
out-projection.

SPMD trick: one program runs on all 8 cores, so per-core differences
live in the DATA only. x rows are fed in a per-core storage permutation
that puts each core's owned query chunks at uniform offsets (storage
chunks 0,2,4,6). Causal masking splits into a role-independent part and
a data part: the diagonal k-chunk (storage chunk 2g of slot g) gets a
compile-time triangular mask (slices of one "staircase" tile) applied
over the valid q-suffix only; the other boundary chunk is all-valid or
all-masked per core, folded into the exp as a per-partition bias of
0/-30 (exp(-30)~=0) -- no per-core mask tensors at all.

x is passed D-major AND bf16 (cast on host: halves the HBM read); the
output is written bf16 (upcast on host). All transposes run on the
tensor engine.

Pipelining: emission is woven so the in-order engine queues interleave
next-chunk K/V/Q projection matmuls into the exp-wait bubbles of the
attention block stream; PSUM is one shared 1-bank pool (scores /
projections / transposes / out-proj, 4 banks) + 4 rotating po banks.

Layouts (partition dim first):
  xT   [128, 8, 4096]  bf16   x^T per d-chunk
  K^T  [128, 4096]     bf16   H-major keys
  Q^T  [128, 2048]     bf16   H-major owned queries
  V    [128, 32, 256]  bf16   token-major V tiles (PE-transposed from the
                              H-major projection); col 128 = ones column
                              for the fused rowsum trick
  scores_T [k=128, q<=512] PSUM; P_T = exp(scale*s + bias) bf16 (ACT)
  O [q=128, 128+1] accumulates in PSUM over k-blocks with P_T subtiles as
  the stationary operand and [V|1] moving; col 128 = softmax denominator.
  Out-proj: lhsT = O^T tile (PE transpose), rhs = Wo^T; the 1/denominator
  scale rides the PSUM->SBUF copy (tensor_scalar_mul on DVE for one half,
  ACT copy-with-scale for the other).
"""

import numpy as np
import ml_dtypes
from contextlib import ExitStack

import concourse.bass as bass
import concourse.tile as tile
from concourse import bacc, mybir
from concourse.bass_utils import run_bass_kernel_spmd

S, B, D, H = 4096, 4, 1024, 128
P = 128
QC = 512                  # query chunk
NSLOT = 4                 # owned chunks per core
DC = D // P               # 8 d-chunks
TT = S // P               # 32 token tiles / k-blocks
NKT = S // QC             # 8 key 512-chunks
SCALE = float(H) ** -0.5
UW = 896                  # staircase width: 384 zero-pad + 512 triangle

# storage-order permutation of the 8 query chunks, per role. Queries the
# core owns sit at storage chunks 0,2,4,6; the first 2(g+1) storage
# chunks cover every true key needed by owned chunk g (extras masked).
SIGMA = {0: [0, 1, 3, 2, 4, 5, 7, 6], 1: [1, 0, 2, 3, 5, 4, 6, 7]}
QSLOT = [0, 2, 4, 6]      # storage chunk positions of owned queries

F32 = mybir.dt.float32
BF16 = mybir.dt.bfloat16


def _build_kernel():
    nc = bacc.Bacc("TRN2", target_bir_lowering=False, debug=False, num_devices=8)

    xbT = nc.dram_tensor("xbT", [D, S], BF16, kind="ExternalInput")
    wqT = nc.dram_tensor("wqT", [P, DC, H], BF16, kind="ExternalInput")
    wkT = nc.dram_tensor("wkT", [P, DC, H], BF16, kind="ExternalInput")
    wvT = nc.dram_tensor("wvT", [P, DC, H], BF16, kind="ExternalInput")
    woT = nc.dram_tensor("woT", [H, D], BF16, kind="ExternalInput")
    bias = nc.dram_tensor("bias", [P, NSLOT * 8], F32, kind="ExternalInput")
    out = nc.dram_tensor("out", [NSLOT * QC, D], BF16, kind="ExternalOutput")

    with ExitStack() as ctx:
        tc = ctx.enter_context(tile.TileContext(nc))
        _body(ctx, tc, xbT.ap(), wqT.ap(), wkT.ap(), wvT.ap(), woT.ap(),
              bias.ap(), out.ap())

    nc.compile()
    return nc


def _interleave(a_units, b_units):
    """Emit a-units with b-units spread evenly between them (b slightly
    front-loaded so woven work finishes before the a-stream does)."""
    if not a_units:
        for b in b_units:
            b()
        return
    ratio = len(b_units) / len(a_units)
    acc = 0.0
    bi = 0
    for a in a_units:
        a()
        acc += ratio
        while bi < len(b_units) and acc >= 1.0:
            b_units[bi]()
            bi += 1
            acc -= 1.0
    while bi < len(b_units):
        b_units[bi]()
        bi += 1


def _body(ctx, tc, xbT, wqT, wkT, wvT, woT, bias, out):
    nc = tc.nc

    consts = ctx.enter_context(tc.tile_pool(name="consts", bufs=1))
    bigbuf = ctx.enter_context(tc.tile_pool(name="bigbuf", bufs=1))
    ptpool = ctx.enter_context(tc.tile_pool(name="pt", bufs=8))
    otmp_pool = ctx.enter_context(tc.tile_pool(name="otmp", bufs=6))
    ypool = ctx.enter_context(tc.tile_pool(name="y", bufs=4))
    # One shared 1-bank pool for scores / projections / transposes /
    # out-proj (4 banks) + 4 rotating po accumulator banks = 8 banks.
    pspool = ctx.enter_context(tc.tile_pool(name="ps", bufs=6, space="PSUM"))
    psO = ctx.enter_context(tc.tile_pool(name="psO", bufs=1, space="PSUM"))

    # ---- constants (cast f32 -> bf16 in the SWDGE DMA) ----
    wq_sb = consts.tile([P, DC, H], BF16)
    wk_sb = consts.tile([P, DC, H], BF16)
    wv_sb = consts.tile([P, DC, H], BF16)
    woT_sb = consts.tile([P, D], BF16)
    bias_sb = consts.tile([P, NSLOT * 8], F32)

    xT = bigbuf.tile([P, DC, S], BF16)
    k_sb = bigbuf.tile([P, S], BF16)
    vT_sb = bigbuf.tile([P, S], BF16)
    q_sb = bigbuf.tile([P, NSLOT * QC], BF16)
    v_sb = bigbuf.tile([P, TT, 2 * P], BF16)  # V k-blocks + ones col (padded stride)
    o_t = bigbuf.tile([P, NSLOT * NSLOT, P], BF16)  # O^T [h, q-tile, q], unnorm
    rec_sb = bigbuf.tile([P, NSLOT * NSLOT], F32)   # 1/rowsum per q-tile column

    nc.gpsimd.dma_start(wk_sb[:], wkT)
    nc.sync.dma_start(wv_sb[:], wvT)
    nc.sync.dma_start(wq_sb[:], wqT)
    # x chunk 0 then chunk 1, one [128, 512] DMA per d-chunk, issue
    # streams striped across the gpsimd and sync queues so descriptor
    # generation parallelizes (projection chains consume c in order).
    for kt in range(2):
        for c in range(DC):
            (nc.gpsimd if c % 2 == 0 else nc.sync).dma_start(
                xT[:, c, bass.ts(kt, QC)], xbT[bass.ts(c, P), bass.ts(kt, QC)])
        if kt == 0:
            nc.gpsimd.dma_start(bias_sb[:], bias)
    nc.gpsimd.dma_start(woT_sb[:], woT)

    # Staircase tile: univ[k, x] = 1.0 where x - 384 >= k. The triangular
    # mask for diagonal k-block j is univ[:, 384 : 896-128j] over the
    # valid q-suffix, and the 128x128 identity (for PE transposes) is
    # univ[:, 384:512] - univ[:, 383:511]. Built BEFORE the bulk x stream
    # so the gpsimd engine isn't stuck behind 24 DMA descriptor-gens.
    stair = consts.tile([P, UW], F32)
    nc.gpsimd.iota(stair[:], pattern=[[1, UW]], base=-384, channel_multiplier=-1,
                   allow_small_or_imprecise_dtypes=True)
    univ = consts.tile([P, UW], BF16)
    nc.vector.tensor_scalar(univ[:], stair[:], 0.0, None,
                            op0=mybir.AluOpType.is_ge)
    ident = consts.tile([P, P], BF16)
    nc.vector.tensor_sub(ident[:], univ[:, 384 : 384 + P], univ[:, 383 : 383 + P])
    nc.vector.memset(v_sb[:, :, H], 1.0)  # ones column for rowsum trick

    # stream the rest of x up front: the DGE queue drains continuously
    # instead of bursting at slot boundaries (consumers dep on pair DMAs)
    for c0 in range(2, NKT, 2):
        for c in range(DC):
            nc.gpsimd.dma_start(xT[:, c, bass.ds(c0 * QC, 2 * QC)],
                                xbT[bass.ts(c, P), bass.ds(c0 * QC, 2 * QC)])

    # ---------- emission units ----------
    def proj_units(w_sb, dst, src_kt, dst_kt=None, pool=None, pname="ps"):
        """Project one 512-token chunk through one weight: 4 units of
        2 accumulating matmuls each + a final PSUM->SBUF copy."""
        st = {}
        dkt = src_kt if dst_kt is None else dst_kt
        ppool = pool if pool is not None else pspool

        def unit(i):
            def run():
                if i == 0:
                    st["ps"] = ppool.tile([P, QC], F32, name=pname)
                for c in (2 * i, 2 * i + 1):
                    nc.tensor.matmul(st["ps"][:], lhsT=w_sb[:, c, :],
                                     rhs=xT[:, c, bass.ts(src_kt, QC)],
                                     start=(c == 0), stop=(c == DC - 1))
                if i == 3:
                    nc.vector.tensor_copy(dst[:, bass.ts(dkt, QC)], st["ps"][:])
            return run
        return [unit(i) for i in range(4)]

    def vtr_unit(bk):
        def run():
            pstr = pspool.tile([P, P], BF16, name="ps")
            nc.tensor.transpose(pstr[:], vT_sb[:, bass.ts(bk, P)], ident[:])
            nc.vector.tensor_copy(v_sb[:, bk, 0:H], pstr[:])
        return run

    po_tiles = {}  # slot -> (po3 bank: subs 0-2, po1 bank: sub 3)

    def po_ap(g, sub):
        po3, po1 = po_tiles[g]
        return po3[:, sub, :] if sub < 3 else po1[:]

    def blk_unit(g, bk, nb):
        def run():
            if bk == 0:
                po_tiles[g] = (psO.tile([P, 3, H + 1], F32, name="po3"),
                               psO.tile([P, H + 1], F32, name="po1"))
            p = bk - 8 * g  # boundary sub-index (>=0 for the last 8 blocks)
            diag_j = p if 0 <= p <= 3 else None
            q0 = 128 * diag_j if diag_j else 0  # valid q-suffix start
            qw = QC - q0
            ps = pspool.tile([P, QC], F32, name="ps")
            nc.tensor.matmul(ps[:, q0:], lhsT=k_sb[:, bass.ts(bk, P)],
                             rhs=q_sb[:, g * QC + q0 : (g + 1) * QC],
                             start=True, stop=True)
            pt = ptpool.tile([P, QC], BF16)
            bias_ap = 0.0
            if p >= 4:  # off-diagonal boundary chunk: all-or-nothing per core
                bias_ap = bias_sb[:, g * 8 + p : g * 8 + p + 1]
            nc.scalar.activation(pt[:, q0:], ps[:, q0:],
                                 mybir.ActivationFunctionType.Exp,
                                 bias=bias_ap, scale=SCALE)
            if diag_j is not None:  # compile-time triangle over the suffix
                nc.vector.tensor_mul(pt[:, q0:], pt[:, q0:],
                                     univ[:, 384 : UW - 128 * diag_j])
            # Three accumulation groups share the po3 bank: sub 0's
            # start=True clears the whole bank's has_written bits, so
            # subs 1,2 start with start=False (overwrite-on-clear-bit).
            for sub in range(diag_j or 0, NSLOT):
                nc.tensor.matmul(po_ap(g, sub), lhsT=pt[:, bass.ts(sub, P)],
                                 rhs=v_sb[:, bk, 0 : H + 1],
                                 start=(bk == 0 and sub in (0, 3)),
                                 stop=(bk == nb - 1),
                                 skip_group_check=(sub in (1, 2)))
        return run

    def finish_unit(g, sub):
        def run():
            po = po_ap(g, sub)
            idx = g * NSLOT + sub
            nc.vector.reciprocal(rec_sb[:, idx : idx + 1], po[:, H : H + 1])
            ob = otmp_pool.tile([P, P], BF16, name="ob")
            nc.vector.tensor_copy(ob[:], po[:, 0:H])
            pstr = pspool.tile([P, P], BF16, name="ps")
            nc.tensor.transpose(pstr[:], ob[:], ident[:])
            nc.vector.tensor_copy(o_t[:, idx, :], pstr[:])
        return run

    def outproj_unit(g, sub):
        tt = g * NSLOT + sub

        def run():
            y = ypool.tile([P, D], BF16)
            for half in range(2):
                ps = pspool.tile([P, QC], F32, name="ps")
                nc.tensor.matmul(ps[:], lhsT=o_t[:, tt, :],
                                 rhs=woT_sb[:, bass.ts(half, QC)],
                                 start=True, stop=True)
                if half == 0:  # split the normalize-copies across DVE and ACT
                    nc.vector.tensor_scalar_mul(y[:, bass.ts(half, QC)], ps[:],
                                                rec_sb[:, tt : tt + 1])
                else:
                    nc.scalar.mul(y[:, bass.ts(half, QC)], ps[:],
                                  rec_sb[:, tt : tt + 1])
                nc.sync.dma_start(out[bass.ts(tt, P), bass.ts(half, QC)],
                                  y[:, bass.ts(half, QC)])
        return run

    # ---------- prologue: chunks 0,1 projected; Q for slot 0 ----------
    # Five projection chains drained c-major so each chain consumes every
    # d-chunk right as its DMA lands (the 5th chain borrows a psO bank).
    chains = [proj_units(wk_sb, k_sb, 0),
              proj_units(wv_sb, vT_sb, 0),
              proj_units(wq_sb, q_sb, 0, dst_kt=0),
              proj_units(wk_sb, k_sb, 1),
              proj_units(wv_sb, vT_sb, 1, pool=psO, pname="po3")]
    for j in range(4):
        for ch in chains:
            ch[j]()
    for bk in range(8):
        vtr_unit(bk)()

    # ---------- main: slot-major weave ----------
    for g in range(NSLOT):
        c0 = 2 * g + 2  # chunk pair projected during this slot (preloaded)
        nb = 8 * (g + 1)
        a_units = [blk_unit(g, bk, nb) for bk in range(nb)]
        b_units = []
        if g > 0:
            b_units += [finish_unit(g - 1, s) for s in range(NSLOT)]
            b_units += [outproj_unit(g - 1, s) for s in range(NSLOT)]
        if c0 < NKT:
            b_units += (proj_units(wk_sb, k_sb, c0)
                        + proj_units(wv_sb, vT_sb, c0)
                        + [vtr_unit(bk) for bk in range(4 * c0, 4 * c0 + 4)]
                        + proj_units(wq_sb, q_sb, c0, dst_kt=g + 1)
                        + proj_units(wk_sb, k_sb, c0 + 1)
                        + proj_units(wv_sb, vT_sb, c0 + 1)
                        + [vtr_unit(bk) for bk in range(4 * c0 + 4, 4 * c0 + 8)])
        _interleave(a_units, b_units)

    for s in range(NSLOT):
        finish_unit(NSLOT - 1, s)()
    for s in range(NSLOT):
        outproj_unit(NSLOT - 1, s)()


_CACHED_NC = None


def _get_nc():
    global _CACHED_NC
    if _CACHED_NC is None:
        _CACHED_NC = _build_kernel()
    return _CACHED_NC


def _make_core_inputs(x, wqT, wkT, wvT, woT, core):
    b, role = core // 2, core % 2
    sigma = SIGMA[role]
    perm = np.concatenate([np.arange(QC) + c * QC for c in sigma])
    xbT = np.ascontiguousarray(x[perm, b, :].T.astype(ml_dtypes.bfloat16))

    # exp bias per boundary block: diagonal chunk (p<4) is handled by the
    # compile-time triangle; the other boundary chunk (p>=4) is all-valid
    # (0) or all-masked (-30 => exp~=0) depending on the role's zigzag.
    bias = np.zeros((P, NSLOT * 8), np.float32)
    for g in range(NSLOT):
        c_q = sigma[2 * g]
        c_k = sigma[2 * g + 1]
        if c_k > c_q:
            bias[:, g * 8 + 4 : g * 8 + 8] = -30.0
    return {"xbT": xbT, "wqT": wqT, "wkT": wkT, "wvT": wvT, "woT": woT,
            "bias": bias}


def _w_pch(w):
    """(H, D) weight -> [p, c, h] bf16 layout for a contiguous SBUF load."""
    return np.ascontiguousarray(
        np.asarray(w, np.float32).T.reshape(DC, P, H).transpose(1, 0, 2)
        .astype(ml_dtypes.bfloat16))


def kernel(x, Wq, Wk, Wv, Wo):
    x = np.asarray(x, dtype=np.float32)
    wqT = _w_pch(Wq)
    wkT = _w_pch(Wk)
    wvT = _w_pch(Wv)
    woT = np.ascontiguousarray(np.asarray(Wo, np.float32).T.astype(ml_dtypes.bfloat16))

    nc = _get_nc()
    in_maps = [_make_core_inputs(x, wqT, wkT, wvT, woT, i) for i in range(8)]
    res = run_bass_kernel_spmd(nc, in_maps, list(range(8))).results

    out = np.empty((S, B, D), np.float32)
    for core in range(8):
        b, role = core // 2, core % 2
        sigma = SIGMA[role]
        co = np.asarray(res[core]["out"], np.float32)
        for g in range(NSLOT):
            c_g = sigma[QSLOT[g]]
            out[c_g * QC : (c_g + 1) * QC, b, :] = co[g * QC : (g + 1) * QC, :]
    return out
